# revision 22
# baseline (speedup 1.0000x reference)
"""Bass/Trainium2 kernel for the listener-speller attention module.

Math restructure (validated to ~5e-3 rel err against the reference,
gate is 2e-2):
  query  = speller_state @ Wq + bq                      (host, tiny)
  qk     = query @ Wk.T                                 (host, tiny)
  hq     = bf16(h * qk)  (broadcast over l)             (host, one pass)
  energy = sum_d hq                                     (device, DVE+ACT)
  wu     = exp(energy) * mask  (bf16)                   (device, ACT+DVE)
  cd'    = sum_l wu[l] * hq[l, :]  (= qk .* cd)         (device, PE bf16)
  cd     = cd' / qk                                     (host, tiny)
  su     = sum_l wu[l]                                  (host, tiny)
  w      = wu / su                                      (host, tiny)
  ctx    = (cd / su) @ Wv + (sum w) * bv                (host, tiny)

Folding qk into the streamed tensor means the device consumes ONE bf16
input (16 MB/core) for both the energy reduction and the context
accumulation; the per-d scale qk[d] this introduces into cd is divided
out on the host (a constant scale on an accumulation leaves relative
error unchanged; qk has no exact zeros, min |qk| ~ 6e-6).

The softmax max-shift is dropped: softmax is shift invariant and the L1
renorm cancels the softmax denominator, so w == exp(e)*mask / sum.
Energies are bounded (|e| < ~15 needs 6+ sigma), far from f32 exp
overflow at 88.

Device reads hq exactly once — the memory-bound part. Data-parallel
over the batch dim N=64 -> 8 cores x 8 rows.

Per-n engine split (hq[n] tiled as [128 part, 16 chunks x 512]):
  PE : 16 accumulating [128,1].T @ [128,512] bf16 matmuls for cd'.
  ACT: act_direct chunk sums via activation(Identity, accum_out), Exp,
       the cd' PSUM->SBUF copy.
  DVE: pairwise tree sum for the remaining chunks (bf16 adds, last
       level f32, then one reduce), mask multiply.
"""

import numpy as np
import ml_dtypes
from contextlib import ExitStack

import concourse.bacc as bacc
import concourse.tile as tile
from concourse import mybir
from concourse.bass_utils import run_bass_kernel_spmd
from concourse.tile_rust import add_dep_helper

F32 = mybir.dt.float32
BF16 = mybir.dt.bfloat16
NP_BF16 = ml_dtypes.bfloat16
N_CORES = 8
N_PER = 8  # batch rows per core
L = 2048
D = 512
P = 128
C = L // P  # 16 l-chunks of 128

_NC_CACHE = None


def build_nc(h_bufs=8, act_direct=3, tree_levels=3, h_split=2, R=1,
             dma_chain=2):
    """act_direct: chunks 0..act_direct-1 of each row summed by ACT directly
    from hq; the rest go through a DVE pairwise tree. R: batch rows packed
    per SBUF tile (tree/exp/mask ops cover R rows per instruction)."""
    assert N_PER % R == 0
    nc = bacc.Bacc(
        "TRN2", target_bir_lowering=False, debug=False, num_devices=N_CORES
    )
    h = nc.declare_dram_parameter("hq", [N_PER, P, C * D], BF16, isOutput=False)
    mask = nc.declare_dram_parameter("mask", [P, N_PER * C], BF16, isOutput=False)
    wu = nc.declare_dram_parameter("wu", [P, N_PER * C], BF16, isOutput=True)
    cd = nc.declare_dram_parameter("cd", [N_PER, D], F32, isOutput=True)

    with tile.TileContext(nc) as tc:
        with ExitStack() as ctx:
            const_pool = ctx.enter_context(tc.tile_pool(name="const", bufs=1))
            hpool = ctx.enter_context(tc.tile_pool(name="h", bufs=h_bufs))
            spool = ctx.enter_context(tc.tile_pool(name="stage", bufs=2))
            small = ctx.enter_context(tc.tile_pool(name="small", bufs=3))
            junk_pool = ctx.enter_context(tc.tile_pool(name="junk", bufs=1))
            cdp_pool = ctx.enter_context(
                tc.tile_pool(name="cdp", bufs=2, space="PSUM")
            )

            mask_all = const_pool.tile([P, N_PER * C], BF16)
            nc.sync.dma_start(out=mask_all[:], in_=mask.ap())

            ajunk = junk_pool.tile([P, D], BF16)
            wum_all = const_pool.tile([P, N_PER * C], BF16)
            cds_all = const_pool.tile([1, N_PER * D], F32)

            a0 = act_direct
            ct = C - act_direct  # chunks through the DVE tree, per row
            row_pushes = {}  # row -> (first_push, last_push)
            for pn in range(N_PER // R):
                # R rows of hq packed in one tile: [p, (r c d)]
                ht = hpool.tile([P, R * C * D], BF16, tag="ht")
                for r in range(R):
                    n = pn * R + r
                    hv = h.ap()[n]
                    step = (C // h_split) * D
                    pushes = []
                    for s in range(h_split):
                        pushes.append(
                            nc.sync.dma_start(
                                out=ht[
                                    :,
                                    r * C * D
                                    + s * step : r * C * D
                                    + (s + 1) * step,
                                ],
                                in_=hv[:, s * step : (s + 1) * step],
                            )
                        )
                    row_pushes[n] = (pushes[0], pushes[-1])
                    if dma_chain and n >= dma_chain:
                        # bound rows in flight so early rows get full HBM
                        # bandwidth and arrive in order (shorter ramp)
                        add_dep_helper(
                            row_pushes[n][0].ins
                            if hasattr(row_pushes[n][0], "ins")
                            else row_pushes[n][0],
                            row_pushes[n - dma_chain][1].ins
                            if hasattr(row_pushes[n - dma_chain][1], "ins")
                            else row_pushes[n - dma_chain][1],
                            reason="dma row pacing",
                        )

                # energy: e[p, (r c)] = sum_d hq[p, r, c, d]
                e = small.tile([P, R * C], F32, tag="e")
                for r in range(R):
                    for c in range(act_direct):
                        nc.scalar.activation(
                            ajunk[:],
                            ht[:, (r * C + c) * D : (r * C + c + 1) * D],
                            mybir.ActivationFunctionType.Identity,
                            accum_out=e[:, r * C + c : r * C + c + 1],
                        )
                if ct:
                    src = ht[:].rearrange("p (r c d) -> p r c d", r=R, d=D)[
                        :, :, a0:C, :
                    ]
                    width = D
                    for lev in range(tree_levels):
                        width //= 2
                        last = lev == tree_levels - 1
                        st = spool.tile(
                            [P, R * ct * width],
                            F32 if last else BF16,
                            tag=f"s{lev}",
                        )
                        st4 = st[:].rearrange(
                            "p (r c d) -> p r c d", r=R, d=width
                        )
                        nc.vector.tensor_add(
                            st4,
                            src[:, :, :, 0:width],
                            src[:, :, :, width : 2 * width],
                        )
                        src = st4
                    nc.vector.tensor_reduce(
                        e[:].rearrange("p (r c) -> p r c", r=R)[:, :, a0:C],
                        src,
                        mybir.AxisListType.X,
                        mybir.AluOpType.add,
                    )

                wue = small.tile([P, R * C], BF16, tag="wue")
                nc.scalar.activation(
                    wue[:], e[:], mybir.ActivationFunctionType.Exp
                )
                wum = wum_all[:, pn * R * C : (pn + 1) * R * C]
                nc.vector.tensor_mul(
                    wum, wue[:], mask_all[:, pn * R * C : (pn + 1) * R * C]
                )

                # cd'[d] = sum_l wu[l] hq[l, d]: 16 accumulating bf16 matmuls
                for r in range(R):
                    n = pn * R + r
                    cdp = cdp_pool.tile([1, D], F32, tag="cdp")
                    for c in range(C):
                        nc.tensor.matmul(
                            cdp[:],
                            wum[:, r * C + c : r * C + c + 1],
                            ht[:, (r * C + c) * D : (r * C + c + 1) * D],
                            start=(c == 0),
                            stop=(c == C - 1),
                        )
                    nc.scalar.copy(cds_all[:, n * D : (n + 1) * D], cdp[:])

            # batched outputs: one wu DMA, one cd DMA
            nc.sync.dma_start(out=wu.ap(), in_=wum_all[:])
            nc.sync.dma_start(
                out=cd.ap().rearrange("n d -> (n d)")[None, :], in_=cds_all[:]
            )

    nc.compile()
    return nc


def _get_nc():
    global _NC_CACHE
    if _NC_CACHE is None:
        _NC_CACHE = build_nc()
    return _NC_CACHE


def host_prep(inputs):
    h = np.asarray(inputs["listener_hiddens"], dtype=np.float32)
    sp = np.asarray(inputs["speller_state"], dtype=np.float32)
    ll = np.asarray(inputs["listener_len"])
    Wk = np.asarray(inputs["Wk"], dtype=np.float32)
    Wq = np.asarray(inputs["Wq"], dtype=np.float32)
    bq = np.asarray(inputs["bq"], dtype=np.float32)
    query = sp @ Wq + bq
    qk = np.ascontiguousarray((query @ Wk.T).astype(np.float32))  # (N, D)
    hq = (h * qk[:, None, :]).astype(NP_BF16)
    N = h.shape[0]
    # [n, l, d] -> [n, p, c*d] with l = c*128 + p, so each partition's DMA
    # source is one contiguous 16KB run (few fat descriptors, cheap HWDGE push)
    hq = np.ascontiguousarray(
        hq.reshape(N, C, P, D).transpose(0, 2, 1, 3).reshape(N, P, C * D)
    )
    maskf = (np.arange(L)[None, :] < ll[:, None]).astype(NP_BF16)  # (N, L)
    # device-native [P, N*C] layout: mask[p, n*C + c] = maskf[n, c*128 + p]
    mask_pc = np.ascontiguousarray(
        maskf.reshape(N, C, P).transpose(2, 0, 1).reshape(P, N * C)
    )
    return hq, qk, mask_pc


def host_post(wu, cdp, qk, inputs):
    """wu: (N, L) unnormalized masked exp; cdp: (N, D) = qk .* cd."""
    Wv = np.asarray(inputs["Wv"], dtype=np.float32)
    bv = np.asarray(inputs["bv"], dtype=np.float32)
    qk_safe = np.where(np.abs(qk) < 1e-30, 1.0, qk)
    cdv = cdp / qk_safe
    su = np.maximum(wu.sum(axis=1, dtype=np.float64), 1e-300)
    w = (wu / su[:, None]).astype(np.float32)
    sw = w.sum(axis=1)
    context = ((cdv / su[:, None]).astype(np.float32) @ Wv + sw[:, None] * bv).astype(
        np.float32
    )
    return context, w


def kernel(**inputs):
    hq, qk, mask_pc = host_prep(inputs)
    nc = _get_nc()
    in_maps = []
    for g in range(N_CORES):
        sl = slice(N_PER * g, N_PER * (g + 1))
        in_maps.append(
            {"hq": hq[sl], "mask": mask_pc[:, N_PER * C * g : N_PER * C * (g + 1)]}
        )
    res = run_bass_kernel_spmd(nc, in_maps, core_ids=list(range(N_CORES))).results
    wu = np.concatenate(
        [
            r["wu"]
            .astype(np.float32)
            .reshape(P, N_PER, C)
            .transpose(1, 2, 0)
            .reshape(N_PER, L)
            for r in res
        ],
        axis=0,
    )
    cdp = np.concatenate([r["cd"].astype(np.float32) for r in res], axis=0)
    return host_post(wu, cdp, qk, inputs)


# revision 24
# speedup vs baseline: 1.0095x; 1.0095x over previous
"""Bass/Trainium2 kernel for the listener-speller attention module.

Math restructure (validated to ~5e-3 rel err against the reference,
gate is 2e-2):
  query  = speller_state @ Wq + bq                      (host, tiny)
  qk     = query @ Wk.T                                 (host, tiny)
  hq     = bf16(h * qk)  (broadcast over l)             (host, one pass)
  energy = sum_d hq                                     (device, DVE+ACT)
  wu     = exp(energy) * mask  (bf16)                   (device, ACT+DVE)
  cd'    = sum_l wu[l] * hq[l, :]  (= qk .* cd)         (device, PE bf16)
  cd     = cd' / qk                                     (host, tiny)
  su     = sum_l wu[l]                                  (host, tiny)
  w      = wu / su                                      (host, tiny)
  ctx    = (cd / su) @ Wv + (sum w) * bv                (host, tiny)

Folding qk into the streamed tensor means the device consumes ONE bf16
input (16 MB/core) for both the energy reduction and the context
accumulation; the per-d scale qk[d] this introduces into cd is divided
out on the host (a constant scale on an accumulation leaves relative
error unchanged; qk has no exact zeros, min |qk| ~ 6e-6).

The softmax max-shift is dropped: softmax is shift invariant and the L1
renorm cancels the softmax denominator, so w == exp(e)*mask / sum.
Energies are bounded (|e| < ~15 needs 6+ sigma), far from f32 exp
overflow at 88.

Device reads hq exactly once — the memory-bound part. Data-parallel
over the batch dim N=64 -> 8 cores x 8 rows.

Per-n engine split (hq[n] tiled as [128 part, 16 chunks x 512]):
  PE : 16 accumulating [128,1].T @ [128,512] bf16 matmuls for cd'.
  ACT: act_direct chunk sums via activation(Identity, accum_out), Exp,
       the cd' PSUM->SBUF copy.
  DVE: pairwise tree sum for the remaining chunks (bf16 adds, last
       level f32, then one reduce), mask multiply.
"""

import numpy as np
import ml_dtypes
from contextlib import ExitStack

import concourse.bacc as bacc
import concourse.tile as tile
from concourse import mybir
from concourse.bass_utils import run_bass_kernel_spmd
from concourse.tile_rust import add_dep_helper

F32 = mybir.dt.float32
BF16 = mybir.dt.bfloat16
NP_BF16 = ml_dtypes.bfloat16
N_CORES = 8
N_PER = 8  # batch rows per core
L = 2048
D = 512
P = 128
C = L // P  # 16 l-chunks of 128

_NC_CACHE = None


def build_nc(h_bufs=8, act_direct=3, tree_levels=3, h_split=2,
             dma_chain=2, split_rows=()):
    """act_direct: chunks 0..act_direct-1 of each row summed by ACT directly
    from hq; the rest go through a DVE pairwise tree. Rows in split_rows are
    processed as two independent half-row segments: the first row's tree can
    start after half the DMA (shorter ramp) and the last row's post-DMA
    compute chain halves (shorter tail). Steady-state rows stay whole."""
    nc = bacc.Bacc(
        "TRN2", target_bir_lowering=False, debug=False, num_devices=N_CORES
    )
    h = nc.declare_dram_parameter("hq", [N_PER, P, C * D], BF16, isOutput=False)
    mask = nc.declare_dram_parameter("mask", [P, N_PER * C], BF16, isOutput=False)
    wu = nc.declare_dram_parameter("wu", [P, N_PER * C], BF16, isOutput=True)
    cd = nc.declare_dram_parameter("cd", [N_PER, D], F32, isOutput=True)

    with tile.TileContext(nc) as tc:
        with ExitStack() as ctx:
            const_pool = ctx.enter_context(tc.tile_pool(name="const", bufs=1))
            hpool = ctx.enter_context(tc.tile_pool(name="h", bufs=h_bufs))
            spool = ctx.enter_context(tc.tile_pool(name="stage", bufs=2))
            small = ctx.enter_context(tc.tile_pool(name="small", bufs=3))
            junk_pool = ctx.enter_context(tc.tile_pool(name="junk", bufs=1))
            cdp_pool = ctx.enter_context(
                tc.tile_pool(name="cdp", bufs=2, space="PSUM")
            )

            mask_all = const_pool.tile([P, N_PER * C], BF16)
            nc.sync.dma_start(out=mask_all[:], in_=mask.ap())

            ajunk = junk_pool.tile([P, D], BF16)
            wum_all = const_pool.tile([P, N_PER * C], BF16)
            cds_all = const_pool.tile([1, N_PER * D], F32)

            def reduce_segment(ht, e, c0, c1):
                """e[:, c] = sum_d ht chunk c, for c in [c0, c1)."""
                for c in range(c0, min(c1, act_direct)):
                    nc.scalar.activation(
                        ajunk[:],
                        ht[:, c * D : (c + 1) * D],
                        mybir.ActivationFunctionType.Identity,
                        accum_out=e[:, c : c + 1],
                    )
                t0 = max(c0, act_direct)
                ct = c1 - t0
                if ct <= 0:
                    return
                src = ht[:].rearrange("p (c d) -> p c d", d=D)[:, t0:c1, :]
                width = D
                for lev in range(tree_levels):
                    width //= 2
                    last = lev == tree_levels - 1
                    st = spool.tile(
                        [P, (C - act_direct) * width],
                        F32 if last else BF16,
                        tag=f"s{lev}",
                    )
                    st3 = st[:].rearrange("p (c d) -> p c d", d=width)[
                        :, 0:ct, :
                    ]
                    nc.vector.tensor_add(
                        st3, src[:, :, 0:width], src[:, :, width : 2 * width]
                    )
                    src = st3
                nc.vector.tensor_reduce(
                    e[:, t0:c1], src, mybir.AxisListType.X, mybir.AluOpType.add
                )

            row_pushes = {}
            for n in range(N_PER):
                segs = [(0, C // 2), (C // 2, C)] if n in split_rows else [(0, C)]
                ht = hpool.tile([P, C * D], BF16, tag="ht")
                hv = h.ap()[n]
                step = (C // h_split) * D
                pushes = []
                for s in range(h_split):
                    pushes.append(
                        nc.sync.dma_start(
                            out=ht[:, s * step : (s + 1) * step],
                            in_=hv[:, s * step : (s + 1) * step],
                        )
                    )
                row_pushes[n] = (pushes[0], pushes[-1])
                if dma_chain and n >= dma_chain:
                    # bound rows in flight so early rows get full HBM
                    # bandwidth and arrive in order (shorter ramp)
                    add_dep_helper(
                        row_pushes[n][0].ins,
                        row_pushes[n - dma_chain][1].ins,
                        reason="dma row pacing",
                    )

                e = small.tile([P, C], F32, tag="e")
                wue = small.tile([P, C], BF16, tag="wue")
                cdp = cdp_pool.tile([1, D], F32, tag="cdp")
                for c0, c1 in segs:
                    reduce_segment(ht, e, c0, c1)
                    nc.scalar.activation(
                        wue[:, c0:c1],
                        e[:, c0:c1],
                        mybir.ActivationFunctionType.Exp,
                    )
                    wum = wum_all[:, n * C + c0 : n * C + c1]
                    nc.vector.tensor_mul(
                        wum,
                        wue[:, c0:c1],
                        mask_all[:, n * C + c0 : n * C + c1],
                    )
                    # cd'[d] += sum_l wu[l] hq[l, d] over this segment
                    for c in range(c0, c1):
                        nc.tensor.matmul(
                            cdp[:],
                            wum_all[:, n * C + c : n * C + c + 1],
                            ht[:, c * D : (c + 1) * D],
                            start=(c == 0),
                            stop=(c == C - 1),
                        )
                nc.scalar.copy(cds_all[:, n * D : (n + 1) * D], cdp[:])

            # batched outputs: one wu DMA, one cd DMA
            nc.sync.dma_start(out=wu.ap(), in_=wum_all[:])
            nc.sync.dma_start(
                out=cd.ap().rearrange("n d -> (n d)")[None, :], in_=cds_all[:]
            )

    nc.compile()
    return nc


def _get_nc():
    global _NC_CACHE
    if _NC_CACHE is None:
        _NC_CACHE = build_nc()
    return _NC_CACHE


def host_prep(inputs):
    h = np.asarray(inputs["listener_hiddens"], dtype=np.float32)
    sp = np.asarray(inputs["speller_state"], dtype=np.float32)
    ll = np.asarray(inputs["listener_len"])
    Wk = np.asarray(inputs["Wk"], dtype=np.float32)
    Wq = np.asarray(inputs["Wq"], dtype=np.float32)
    bq = np.asarray(inputs["bq"], dtype=np.float32)
    query = sp @ Wq + bq
    qk = np.ascontiguousarray((query @ Wk.T).astype(np.float32))  # (N, D)
    hq = (h * qk[:, None, :]).astype(NP_BF16)
    N = h.shape[0]
    # [n, l, d] -> [n, p, c*d] with l = c*128 + p, so each partition's DMA
    # source is one contiguous 16KB run (few fat descriptors, cheap HWDGE push)
    hq = np.ascontiguousarray(
        hq.reshape(N, C, P, D).transpose(0, 2, 1, 3).reshape(N, P, C * D)
    )
    maskf = (np.arange(L)[None, :] < ll[:, None]).astype(NP_BF16)  # (N, L)
    # device-native [P, N*C] layout: mask[p, n*C + c] = maskf[n, c*128 + p]
    mask_pc = np.ascontiguousarray(
        maskf.reshape(N, C, P).transpose(2, 0, 1).reshape(P, N * C)
    )
    return hq, qk, mask_pc


def host_post(wu, cdp, qk, inputs):
    """wu: (N, L) unnormalized masked exp; cdp: (N, D) = qk .* cd."""
    Wv = np.asarray(inputs["Wv"], dtype=np.float32)
    bv = np.asarray(inputs["bv"], dtype=np.float32)
    qk_safe = np.where(np.abs(qk) < 1e-30, 1.0, qk)
    cdv = cdp / qk_safe
    su = np.maximum(wu.sum(axis=1, dtype=np.float64), 1e-300)
    w = (wu / su[:, None]).astype(np.float32)
    sw = w.sum(axis=1)
    context = ((cdv / su[:, None]).astype(np.float32) @ Wv + sw[:, None] * bv).astype(
        np.float32
    )
    return context, w


def kernel(**inputs):
    hq, qk, mask_pc = host_prep(inputs)
    nc = _get_nc()
    in_maps = []
    for g in range(N_CORES):
        sl = slice(N_PER * g, N_PER * (g + 1))
        in_maps.append(
            {"hq": hq[sl], "mask": mask_pc[:, N_PER * C * g : N_PER * C * (g + 1)]}
        )
    res = run_bass_kernel_spmd(nc, in_maps, core_ids=list(range(N_CORES))).results
    wu = np.concatenate(
        [
            r["wu"]
            .astype(np.float32)
            .reshape(P, N_PER, C)
            .transpose(1, 2, 0)
            .reshape(N_PER, L)
            for r in res
        ],
        axis=0,
    )
    cdp = np.concatenate([r["cd"].astype(np.float32) for r in res], axis=0)
    return host_post(wu, cdp, qk, inputs)
